# revision 1
# baseline (speedup 1.0000x reference)
"""Trainium2 Bass kernel for single-head causal attention.

Problem: B=4, T=4096, C=768, fp32.
  Q = x@Wq+bq; K = x@Wk+bk; V = x@Wv+bv
  out = softmax(causal(Q K^T / sqrt(C))) @ V

Sharding (8 cores): 2 cores per batch element. Each core processes ALL 4096
queries of its batch but only HALF the key tiles (128-row tiles, interleaved
by parity m = core%2). This makes the instruction stream identical across
cores (required for SPMD: one NEFF, data-only differences) and splits the
causal flash-attention work exactly 50/50 at i-block granularity of 256.

Each core returns unnormalized O_m = sum_j exp(s_ij) v_j and l_m = sum_j
exp(s_ij) (ones-column trick appended to V). Host combines:
  out = (O_0 + O_1) / (l_0 + l_1) + bv
(bv folds out of the attention average since softmax rows sum to 1;
no max-subtraction needed: |scores| <= ~5 so exp is well-conditioned.)

Matmuls run in float32r (TF32-class, ~1e-4 rel err, full PE rate at
free-dim >= 256). Producers must round to f32r explicitly.
"""
import sys

sys.path.insert(0, "/opt/trn_rl_repo")

import numpy as np
from contextlib import ExitStack

import concourse.bass as bass
import concourse.bacc as bacc
import concourse.mybir as mybir
import concourse.tile as tile
from concourse.bass_utils import run_bass_kernel_spmd
from concourse.masks import make_identity

dt = mybir.dt
F32, F32R = dt.float32, dt.float32r
AFT = mybir.ActivationFunctionType

B, T, C = 4, 4096, 768
NCK = C // 128            # 6 contraction tiles
NKT = T // 2 // 128       # 16 key tiles per core
NQ4 = T // 4              # 1024 queries per quarter-pass
SCALE = 1.0 / float(np.sqrt(np.float32(C)))

_nc_cache = {}
last_exec_time_ns = None
last_results = None


def _transpose_block(nc, ps_tr, xt_dst, x_src, eng_sel, idents):
    ident = idents[0] if x_src.dtype == F32R else idents[1]
    pdt = x_src.dtype
    """PE-transpose x_src [128,768] f32 -> xt_dst view [128, 6, 128] f32r.

    xt_dst is an AP [128, 6, 128] (plane-strided dest). Routes the two
    PSUM evictions to alternating engines via eng_sel (0/1).
    """
    pt = ps_tr.tile([128, 512], pdt, tag="pj", name="trp")
    for k in range(4):
        nc.tensor.matmul(pt[:, 128 * k:128 * k + 128],
                         lhsT=x_src[:, 128 * k:128 * k + 128], rhs=ident[:],
                         is_transpose=True, start=(k == 0), stop=(k == 3))
    if eng_sel == 0:
        nc.scalar.activation(xt_dst[:, 0:4, :], pt[:].rearrange("p (k f) -> p k f", k=4), AFT.Identity)
    else:
        nc.vector.tensor_copy(xt_dst[:, 0:4, :], pt[:].rearrange("p (k f) -> p k f", k=4))
    pt2 = ps_tr.tile([128, 512], pdt, tag="pj", name="trp2")
    for k in range(2):
        nc.tensor.matmul(pt2[:, 128 * k:128 * k + 128],
                         lhsT=x_src[:, 512 + 128 * k:512 + 128 * k + 128], rhs=ident[:],
                         is_transpose=True, start=(k == 0), stop=(k == 1))
    if eng_sel == 0:
        nc.vector.tensor_copy(xt_dst[:, 4:6, :], pt2[:, 0:256].rearrange("p (k f) -> p k f", k=2))
    else:
        nc.scalar.activation(xt_dst[:, 4:6, :], pt2[:, 0:256].rearrange("p (k f) -> p k f", k=2), AFT.Identity)


def build_module():
    nc = bacc.Bacc("TRN2", target_bir_lowering=False, debug=False)

    xq = nc.dram_tensor("xq", [T, C], F32, kind="ExternalInput").ap()
    xk = nc.dram_tensor("xk", [T // 2, C], F32, kind="ExternalInput").ap()
    wq = nc.dram_tensor("wq", [C, C], F32, kind="ExternalInput").ap()
    wk = nc.dram_tensor("wk", [C, C], F32, kind="ExternalInput").ap()
    wv = nc.dram_tensor("wv", [C, C], F32, kind="ExternalInput").ap()
    bq = nc.dram_tensor("bq", [C], F32, kind="ExternalInput").ap()
    bk = nc.dram_tensor("bk", [C], F32, kind="ExternalInput").ap()
    msk = nc.dram_tensor("msk", [128, 256], F32, kind="ExternalInput").ap()
    out = nc.dram_tensor("out", [T, C + 1], F32, kind="ExternalOutput").ap()

    with tile.TileContext(nc) as tc, ExitStack() as ctx:
        const = ctx.enter_context(tc.tile_pool(name="const", bufs=1))
        ident_f = const.tile([128, 128], F32)
        ident = const.tile([128, 128], F32R)
        mask_sb = const.tile([128, 256], F32R)
        bq_sb = const.tile([128, NCK], F32)
        nc.sync.dma_start(bq_sb[:], bq.rearrange("(k p) -> p k", p=128))
        bk_sb = const.tile([128, NCK], F32)
        nc.sync.dma_start(bk_sb[:], bk.rearrange("(k p) -> p k", p=128))
        onez = const.tile([128, 2], F32)
        nc.vector.memset(onez[:, 0:1], 1.0)
        nc.vector.memset(onez[:, 1:2], 0.0)

        # --- weights: load fp32, round to f32r, plane layout [128, ck, 768]
        wq_pool = ctx.enter_context(tc.tile_pool(name="wq", bufs=1))
        wq_r = wq_pool.tile([128, NCK * C], F32R)
        def stage_w_chunk(wst, w_dram, c, name):
            w_f32 = wst.tile([128, 2 * C], F32, tag="wst", name=name)
            nc.sync.dma_start(
                w_f32[:].rearrange("p (k n) -> p k n", k=2),
                w_dram.rearrange("(k p) n -> p k n", p=128)[:, 2 * c:2 * c + 2, :])
            return w_f32

        # --- PSUM pools (8 banks total):
        # tr: 1 bank, proj: 2, st: 1, O: 4
        kt_pool = ctx.enter_context(tc.tile_pool(name="kt", bufs=1))
        v_pool = ctx.enter_context(tc.tile_pool(name="v", bufs=1))
        # KT planes: [128, dk, 2048]; V tiles: [128, t, 769] (col 768 = ones)
        kt_sb = kt_pool.tile([128, NCK * 2048], F32R)
        v_sb = v_pool.tile([128, NKT * 770], F32R)

        # ---------------- phase K: keys -> KT, V ----------------
        with tc.tile_pool(name="wstage", bufs=2) as wst, \
             tc.tile_pool(name="wkv", bufs=1) as wkv_pool, \
             tc.tile_pool(name="xkst", bufs=6) as xkst, \
             tc.tile_pool(name="xkt", bufs=2) as xktp, \
             tc.tile_pool(name="ps_k", bufs=7, space="PSUM") as ps_k:
            wk_r = wkv_pool.tile([128, NCK * C], F32R)
            wv_r = wkv_pool.tile([128, NCK * C], F32R)
            def cast_w(chunks, w_dst, eng):
                for c, w_f32 in enumerate(chunks):
                    sl = w_dst[:, 2 * C * c: 2 * C * c + 2 * C]
                    if eng == 0:
                        nc.vector.tensor_copy(sl, w_f32[:])
                    else:
                        nc.scalar.activation(sl, w_f32[:], AFT.Identity)

            wk_ch = [stage_w_chunk(wst, wk, c, f"wk{c}") for c in range(3)]
            wv_ch = [stage_w_chunk(wst, wv, c, f"wv{c}") for c in range(3)]
            wq_ch = [stage_w_chunk(wst, wq, c, f"wq{c}") for c in range(3)]
            cast_w(wk_ch, wk_r, 0)
            xk_tiles = {}

            def load_xk(win, tt):
                x_sb = xkst.tile([128, C], F32R, tag="xk", name=f"xk{win}_{tt}")
                nc.gpsimd.dma_start(x_sb[:], xk[512 * win + 128 * tt: 512 * win + 128 * tt + 128, :])
                xk_tiles[(win, tt)] = x_sb

            def emit_tr_k(win):
                xkt = xktp.tile([128, NCK * 512], F32R, tag="xktw", name=f"xkt{win}")
                for tt in range(4):
                    x_sb = xk_tiles.pop((win, tt))
                    xt_view = xkt[:].rearrange("p (k n) -> p k n", k=NCK)[:, :, 128 * tt:128 * tt + 128]
                    _transpose_block(nc, ps_k, xt_view, x_sb, tt % 2, (ident, ident_f))
                    if win + 1 < 4 and (win + 1, tt) not in xk_tiles:
                        load_xk(win + 1, tt)
                return xkt

            for tt in range(4):
                load_xk(0, tt)
            load_xk(1, 0)
            load_xk(1, 1)
            nc.gpsimd.dma_start(mask_sb[:], msk[:])
            make_identity(nc, ident_f[:])
            nc.vector.tensor_copy(ident[:], ident_f[:])
            xkt_cur = emit_tr_k(0)
            for win in range(4):        # 512-key windows
                xkt = xkt_cur
                if win + 1 < 4:
                    xkt_cur = emit_tr_k(win + 1)
                if win == 0:
                    cast_w(wv_ch, wv_r, 1)
                if win == 1:
                    cast_w(wq_ch, wq_r, 0)
                # KT projection for this window: KT[dk, 512*win:+512]
                for co in range(NCK):
                    pj = ps_k.tile([128, 512], F32, tag="pj")
                    for ck in range(NCK):
                        nc.tensor.matmul(
                            pj[:],
                            lhsT=wk_r[:, C * ck + 128 * co: C * ck + 128 * co + 128],
                            rhs=xkt[:, 512 * ck: 512 * ck + 512],
                            start=(ck == 0), stop=(ck == NCK - 1))
                    nc.scalar.activation(kt_sb[:, 2048 * co + 512 * win: 2048 * co + 512 * win + 512],
                                         pj[:], AFT.Identity, bias=bk_sb[:, co:co + 1])
                # V projection for the 4 tiles in this window (no bias)
                for tt in range(4):
                    t_glob = 4 * win + tt
                    for half in range(2):
                        n0, nn = (0, 512) if half == 0 else (512, 256)
                        pj = ps_k.tile([128, 512], F32, tag="pj")
                        for ck in range(NCK):
                            nc.tensor.matmul(
                                pj[:, 0:nn],
                                lhsT=xkt[:, 512 * ck + 128 * tt: 512 * ck + 128 * tt + 128],
                                rhs=wv_r[:, C * ck + n0: C * ck + n0 + nn],
                                start=(ck == 0), stop=(ck == NCK - 1))
                        if half == 0:
                            nc.vector.tensor_copy(v_sb[:, 770 * t_glob: 770 * t_glob + 512], pj[:, 0:512])
                        else:
                            nc.scalar.activation(v_sb[:, 770 * t_glob + 512: 770 * t_glob + 768],
                                                 pj[:, 0:256], AFT.Identity)
                    nc.vector.tensor_copy(v_sb[:, 770 * t_glob + 768: 770 * t_glob + 770], onez[:])

        # ---------------- phase Q: 512-query windows ----------------
        ps_pj = ctx.enter_context(tc.tile_pool(name="ps_pj", bufs=2, space="PSUM"))
        ps_st = ctx.enter_context(tc.tile_pool(name="ps_st", bufs=2, space="PSUM"))
        ps_o = ctx.enter_context(tc.tile_pool(name="ps_o", bufs=1, space="PSUM"))
        with tc.tile_pool(name="xqst", bufs=6) as xqst, \
             tc.tile_pool(name="xqt", bufs=2) as xqtp, \
             tc.tile_pool(name="qt", bufs=2) as qtp, \
             tc.tile_pool(name="pt", bufs=3) as ptp, \
             tc.tile_pool(name="ob", bufs=2) as obp:
            xq_tiles = {}

            QORDER = list(range(7, -1, -1))
            QNEXT = {w: QORDER[i + 1] for i, w in enumerate(QORDER[:-1])}

            def load_xq(widx, tt):
                x_sb = xqst.tile([128, C], F32R, tag="xq", name=f"xq{widx}_{tt}")
                nc.gpsimd.dma_start(x_sb[:], xq[512 * widx + 128 * tt: 512 * widx + 128 * tt + 128, :])
                xq_tiles[(widx, tt)] = x_sb

            def emit_trqt_q(widx):
                qt_sb = qtp.tile([128, NCK * 512], F32R, tag="qt", name=f"qt{widx}")
                xqt = xqtp.tile([128, NCK * 512], F32R, tag="xqtw", name=f"xqt{widx}")
                for tt in range(4):
                    x_sb = xq_tiles.pop((widx, tt))
                    xt_view = xqt[:].rearrange("p (k n) -> p k n", k=NCK)[:, :, 128 * tt:128 * tt + 128]
                    _transpose_block(nc, ps_pj, xt_view, x_sb, tt % 2, (ident, ident_f))
                    if widx in QNEXT:
                        load_xq(QNEXT[widx], tt)
                for co in range(NCK):
                    pj = ps_pj.tile([128, 512], F32, tag="pj")
                    for ck in range(NCK):
                        nc.tensor.matmul(
                            pj[:],
                            lhsT=wq_r[:, C * ck + 128 * co: C * ck + 128 * co + 128],
                            rhs=xqt[:, 512 * ck: 512 * ck + 512],
                            start=(ck == 0), stop=(ck == NCK - 1))
                    nc.scalar.activation(qt_sb[:, 512 * co: 512 * co + 512],
                                         pj[:], AFT.Identity, bias=bq_sb[:, co:co + 1])
                return qt_sb

            for tt in range(4):
                load_xq(QORDER[0], tt)
            for tt in range(2):
                load_xq(QORDER[1], tt)
            qt_next = emit_trqt_q(QORDER[0])
            for wi, widx in enumerate(QORDER):  # big windows first
                qt_sb = qt_next
                if widx in QNEXT:
                    qt_next = emit_trqt_q(QNEXT[widx])
                # flash: i-blocks of 256 (a = 2*widx+al), j-tiles of 128.
                # Software-pipelined emission: ST(k+1) is emitted before the
                # exp-dependent AV(k) so the PE never waits on ACT.
                seq = [(al, t) for al in range(2) for t in range(2 * widx + al + 1)]

                def emit_st(al, t):
                    st = ps_st.tile([128, 256], F32, tag="st", name=f"st{widx}_{al}_{t}")
                    for dk in range(NCK):
                        nc.tensor.matmul(
                            st[:],
                            lhsT=kt_sb[:, 2048 * dk + 128 * t: 2048 * dk + 128 * t + 128],
                            rhs=qt_sb[:, 512 * dk + 256 * al: 512 * dk + 256 * al + 256],
                            start=(dk == 0), stop=(dk == NCK - 1))
                    return st

                st_tiles = {seq[0]: emit_st(*seq[0])}
                o_cur = {}
                for k, (al, t) in enumerate(seq):
                    a = 2 * widx + al
                    if t == 0:
                        o_cur[al] = []
                        for s2 in range(2):
                            oa_t = ps_o.tile([128, 512], F32, tag=f"oa{s2}", name=f"oa{s2}_{a}")
                            ob_t = ps_o.tile([128, 258], F32, tag=f"ob{s2}", name=f"ob{s2}_{a}")
                            o_cur[al].append((oa_t, ob_t))
                    st = st_tiles.pop((al, t))
                    pt = ptp.tile([128, 256], F32R, tag="pt", name=f"pt{widx}_{al}_{t}")
                    nc.scalar.activation(pt[:], st[:], AFT.Exp, scale=SCALE)
                    if t == a:
                        nc.vector.tensor_mul(pt[:], pt[:], mask_sb[:])
                    if k + 1 < len(seq):
                        st_tiles[seq[k + 1]] = emit_st(*seq[k + 1])
                    for s2 in range(2):
                        oa, ob = o_cur[al][s2]
                        nc.tensor.matmul(oa[:], lhsT=pt[:, 128 * s2:128 * s2 + 128],
                                         rhs=v_sb[:, 770 * t:770 * t + 512],
                                         start=(t == 0), stop=(t == a))
                        nc.tensor.matmul(ob[:], lhsT=pt[:, 128 * s2:128 * s2 + 128],
                                         rhs=v_sb[:, 770 * t + 512:770 * t + 770],
                                         start=(t == 0), stop=(t == a))
                    if t == a:
                        for s2 in range(2):
                            oa, ob = o_cur[al][s2]
                            o_sb = obp.tile([128, 770], F32, tag="osb", name=f"osb{a}_{s2}")
                            nc.vector.tensor_copy(o_sb[:, 0:512], oa[:])
                            nc.scalar.activation(o_sb[:, 512:770], ob[:], AFT.Identity)
                            nc.sync.dma_start(out[256 * a + 128 * s2: 256 * a + 128 * s2 + 128, :],
                                              o_sb[:, 0:769])

    nc.compile()
    return nc


def _build_mask(m):
    jl = np.arange(128)[:, None]
    il = np.arange(256)[None, :]
    return (il >= jl + 128 * m).astype(np.float32)


def kernel(input, Wq, bq, Wk, bk, Wv, bv):
    global last_exec_time_ns, last_results
    x = np.ascontiguousarray(np.asarray(input, dtype=np.float32))
    Wq = np.ascontiguousarray(np.asarray(Wq, dtype=np.float32))
    Wk = np.ascontiguousarray(np.asarray(Wk, dtype=np.float32))
    Wv = np.ascontiguousarray(np.asarray(Wv, dtype=np.float32))
    bq = np.ascontiguousarray(np.asarray(bq, dtype=np.float32))
    bk = np.ascontiguousarray(np.asarray(bk, dtype=np.float32))
    bv_np = np.ascontiguousarray(np.asarray(bv, dtype=np.float32))

    if "nc" not in _nc_cache:
        _nc_cache["nc"] = build_module()
    nc = _nc_cache["nc"]

    masks = [_build_mask(m) for m in range(2)]
    key_rows = [np.concatenate([np.arange(128 * (2 * t + m), 128 * (2 * t + m) + 128)
                                for t in range(NKT)]) for m in range(2)]
    in_maps = []
    for core in range(8):
        b, m = core // 2, core % 2
        in_maps.append({
            "xq": x[b],
            "xk": np.ascontiguousarray(x[b][key_rows[m]]),
            "wq": Wq, "wk": Wk, "wv": Wv, "bq": bq, "bk": bk,
            "msk": masks[m],
        })

    trace = bool(int(__import__("os").environ.get("KERNEL_TRACE", "0")))
    res = run_bass_kernel_spmd(nc, in_maps, core_ids=list(range(8)), trace=trace)
    last_exec_time_ns = res.exec_time_ns
    last_results = res

    y = np.empty((B, T, C), dtype=np.float32)
    for b in range(B):
        o0 = res.results[2 * b]["out"]
        o1 = res.results[2 * b + 1]["out"]
        O = o0[:, :C].astype(np.float64) + o1[:, :C].astype(np.float64)
        l = o0[:, C].astype(np.float64) + o1[:, C].astype(np.float64)
        y[b] = (O / l[:, None] + bv_np.astype(np.float64)).astype(np.float32)
    return y



# revision 4
# speedup vs baseline: 1.2600x; 1.2600x over previous
"""Trainium2 Bass kernel for single-head causal attention.

Problem: B=4, T=4096, C=768, fp32.
  Q = x@Wq+bq; K = x@Wk+bk; V = x@Wv+bv
  out = softmax(causal(Q K^T / sqrt(C))) @ V

Sharding (8 cores): 2 cores per batch element. Each core processes ALL 4096
queries of its batch but only HALF the key tiles (128-row tiles, interleaved
by parity m = core%2). Instruction streams are identical across cores (SPMD).

Each core returns unnormalized O_m = sum_j exp(s_ij) v_j and l_m = sum_j
exp(s_ij) (ones-column appended to V). Host combines:
  out = (O_0 + O_1) / (l_0 + l_1) + bv

Datatype strategy (vs the f32r baseline): no f32/f32r matmuls anywhere (f32r
runs the PE in FP32_HIGH mode, which disables the fast-weight-load path and
roughly halves sustained throughput).  x and W ship as bf16; x is transposed
during load by the DMA xbar (no PE transposes).  Projections and attention@V
run bf16.  QT/KT are rounded to fp8e4 and the score matmuls run fp8
DoubleRow (2 contraction tiles per pass, ~2x bf16 rate); softmax
normalization on the host absorbs the fp8 score noise (~1e-2 worst-entry,
vs the 2e-2 gate).
"""
import sys

sys.path.insert(0, "/opt/trn_rl_repo")

import numpy as np
import ml_dtypes
from contextlib import ExitStack

import concourse.bass as bass
import concourse.bacc as bacc
import concourse.mybir as mybir
import concourse.tile as tile
from concourse.bass_utils import run_bass_kernel_spmd

dt = mybir.dt
F32, BF16, FP8 = dt.float32, dt.bfloat16, dt.float8e4
AFT = mybir.ActivationFunctionType
DR = mybir.MatmulPerfMode.DoubleRow

B, T, C = 4, 4096, 768
NCK = C // 128            # 6 contraction tiles
NKT = T // 2 // 128       # 16 key tiles per core
NW = T // 512             # 8 query/key windows of 512
SCALE = 1.0 / float(np.sqrt(np.float32(C)))
ST_FP8 = True             # False: scores in bf16 (more accurate, ~1.2x slower)

_nc_cache = {}
last_exec_time_ns = None
last_results = None


def build_module(st_fp8=ST_FP8):
    qkdt = FP8 if st_fp8 else BF16
    nc = bacc.Bacc("TRN2", target_bir_lowering=False, debug=False)

    xq = nc.dram_tensor("xq", [T, C], BF16, kind="ExternalInput").ap()
    xk = nc.dram_tensor("xk", [T // 2, C], BF16, kind="ExternalInput").ap()
    wqh = nc.dram_tensor("wqh", [C, C], BF16, kind="ExternalInput").ap()
    wkh = nc.dram_tensor("wkh", [C, C], BF16, kind="ExternalInput").ap()
    wvh = nc.dram_tensor("wvh", [C, C], BF16, kind="ExternalInput").ap()
    bq = nc.dram_tensor("bq", [C], F32, kind="ExternalInput").ap()
    bk = nc.dram_tensor("bk", [C], F32, kind="ExternalInput").ap()
    msk = nc.dram_tensor("msk", [128, 1024], BF16, kind="ExternalInput").ap()
    out = nc.dram_tensor("out", [T, C + 1], F32, kind="ExternalOutput").ap()

    with tile.TileContext(nc) as tc, ExitStack() as ctx:
        const = ctx.enter_context(tc.tile_pool(name="const", bufs=1))
        bq_sb = const.tile([128, NCK], F32)
        bk_sb = const.tile([128, NCK], F32)
        mask_sb = const.tile([128, 1024], BF16)
        onez = const.tile([128, 2], BF16)

        # persistent data
        w_pool = ctx.enter_context(tc.tile_pool(name="w", bufs=1))
        wq_b = w_pool.tile([128, NCK * C], BF16)    # planes [p, ck, co]
        wk_b = w_pool.tile([128, NCK * C], BF16)
        wv_b = w_pool.tile([128, NCK * C], BF16)
        kt_pool = ctx.enter_context(tc.tile_pool(name="kt", bufs=1))
        kt8 = kt_pool.tile([128, NCK * 2048], qkdt)  # planes [p, dk, key]
        v_pool = ctx.enter_context(tc.tile_pool(name="v", bufs=1))
        v_b = v_pool.tile([128, NKT * 770], BF16)    # per key tile [128, 770]

        kt3 = kt8[:].rearrange("p (k n) -> p k n", k=NCK)

        nc.sync.dma_start(bq_sb[:], bq.rearrange("(k p) -> p k", p=128))
        nc.sync.dma_start(bk_sb[:], bk.rearrange("(k p) -> p k", p=128))
        nc.sync.dma_start(mask_sb[:], msk[:])
        nc.vector.memset(onez[:, 0:1], 1.0)
        nc.vector.memset(onez[:, 1:2], 0.0)

        def emit_st_mms(stp, lhs3, rhs3, t):
            """Score matmuls for key tile t: out [128 keys, 512 queries]."""
            if st_fp8:
                for j in range(NCK // 2):
                    nc.tensor.matmul(
                        stp[:],
                        lhsT=lhs3[:, 2 * j:2 * j + 2, 128 * t:128 * t + 128],
                        rhs=rhs3[:, 2 * j:2 * j + 2, :],
                        perf_mode=DR, start=(j == 0), stop=(j == NCK // 2 - 1))
            else:
                for j in range(NCK):
                    nc.tensor.matmul(
                        stp[:],
                        lhsT=lhs3[:, j, 128 * t:128 * t + 128],
                        rhs=rhs3[:, j, :],
                        start=(j == 0), stop=(j == NCK - 1))

        # ---------------- phase K: keys -> KT(fp8), V(bf16) ----------------
        with tc.tile_pool(name="xkT", bufs=1) as xkTp, \
             tc.tile_pool(name="ps_k", bufs=2, space="PSUM") as ps_k:
            # weight loads (SWDGE ring, parallel with xbar transposes on sync)
            nc.gpsimd.dma_start(
                wk_b[:].rearrange("p (k n) -> p k n", k=NCK),
                wkh.rearrange("(k p) n -> p k n", p=128))
            nc.gpsimd.dma_start(
                wv_b[:].rearrange("p (k n) -> p k n", k=NCK),
                wvh.rearrange("(k p) n -> p k n", p=128))
            nc.gpsimd.dma_start(
                wq_b[:].rearrange("p (k n) -> p k n", k=NCK),
                wqh.rearrange("(k p) n -> p k n", p=128))

            xkT_b = xkTp.tile([128, NCK * 2048], BF16)  # planes [p, ck, key]

            def emit_xk_tr(kw):
                for k in range(NCK):
                    nc.sync.dma_start(
                        xkT_b[:, 2048 * k + 512 * kw: 2048 * k + 512 * kw + 512],
                        xk[512 * kw: 512 * kw + 512, 128 * k: 128 * k + 128],
                        transpose=True)

            emit_xk_tr(0)
            emit_xk_tr(1)

            for kw in range(4):
                if kw + 2 < 4:
                    emit_xk_tr(kw + 2)
                # K projection: KT[co, keys] bf16, evicted to kt8
                for co in range(NCK):
                    pj = ps_k.tile([128, 512], F32, tag="pj")
                    for ck in range(NCK):
                        nc.tensor.matmul(
                            pj[:],
                            lhsT=wk_b[:, C * ck + 128 * co: C * ck + 128 * co + 128],
                            rhs=xkT_b[:, 2048 * ck + 512 * kw: 2048 * ck + 512 * kw + 512],
                            start=(ck == 0), stop=(ck == NCK - 1))
                    nc.scalar.activation(kt3[:, co, 512 * kw:512 * kw + 512],
                                         pj[:], AFT.Identity, bias=bk_sb[:, co:co + 1])
                # V projection (bf16): per key tile, accumulate over ck
                for tt in range(4):
                    t_glob = 4 * kw + tt
                    pv1 = ps_k.tile([128, 512], F32, tag="pv1")
                    pv2 = ps_k.tile([128, 256], F32, tag="pv2")
                    for ck in range(NCK):
                        lt = xkT_b[:, 2048 * ck + 512 * kw + 128 * tt:
                                   2048 * ck + 512 * kw + 128 * tt + 128]
                        nc.tensor.matmul(pv1[:], lhsT=lt, rhs=wv_b[:, C * ck: C * ck + 512],
                                         start=(ck == 0), stop=(ck == NCK - 1))
                        nc.tensor.matmul(pv2[:], lhsT=lt, rhs=wv_b[:, C * ck + 512: C * ck + 768],
                                         start=(ck == 0), stop=(ck == NCK - 1))
                    nc.vector.tensor_copy(v_b[:, 770 * t_glob: 770 * t_glob + 512], pv1[:])
                    nc.scalar.activation(v_b[:, 770 * t_glob + 512: 770 * t_glob + 768],
                                         pv2[:], AFT.Identity)
                    nc.vector.tensor_copy(v_b[:, 770 * t_glob + 768: 770 * t_glob + 770], onez[:])

        # ---------------- phase Q: flash over 512-query windows ----------------
        ps_pj = ctx.enter_context(tc.tile_pool(name="ps_pj", bufs=2, space="PSUM"))
        ps_st = ctx.enter_context(tc.tile_pool(name="ps_st", bufs=2, space="PSUM"))
        ps_o = ctx.enter_context(tc.tile_pool(name="ps_o", bufs=1, space="PSUM"))
        with tc.tile_pool(name="xqst", bufs=3) as xqst, \
             tc.tile_pool(name="qt", bufs=2) as qtp, \
             tc.tile_pool(name="pt", bufs=16) as ptp, \
             tc.tile_pool(name="ob", bufs=2) as obp:

            QORDER = list(range(NW - 1, -1, -1))  # big windows first

            def emit_xq_tr(w):
                xqt = xqst.tile([128, NCK * 512], BF16, tag="xqt", name=f"xqt{w}")
                for k in range(NCK):
                    nc.sync.dma_start(
                        xqt[:, 512 * k: 512 * k + 512],
                        xq[512 * w: 512 * w + 512, 128 * k: 128 * k + 128],
                        transpose=True)
                return xqt

            def emit_qproj(w, xqt):
                qt_sb = qtp.tile([128, NCK * 512], qkdt, tag="qt", name=f"qt{w}")
                qt3 = qt_sb[:].rearrange("p (k n) -> p k n", k=NCK)
                for co in range(NCK):
                    pj = ps_pj.tile([128, 512], F32, tag="pj")
                    for ck in range(NCK):
                        nc.tensor.matmul(
                            pj[:],
                            lhsT=wq_b[:, C * ck + 128 * co: C * ck + 128 * co + 128],
                            rhs=xqt[:, 512 * ck: 512 * ck + 512],
                            start=(ck == 0), stop=(ck == NCK - 1))
                    nc.scalar.activation(qt3[:, co, :], pj[:], AFT.Identity,
                                         bias=bq_sb[:, co:co + 1])
                return qt_sb

            # stage the first two windows' transposes + first window's proj
            xqt_cache = {QORDER[0]: emit_xq_tr(QORDER[0]),
                         QORDER[1]: emit_xq_tr(QORDER[1])}
            qt_cache = {QORDER[0]: emit_qproj(QORDER[0], xqt_cache.pop(QORDER[0]))}

            for wi, w in enumerate(QORDER):
                qt_sb = qt_cache.pop(w)
                qt3 = qt_sb[:].rearrange("p (k n) -> p k n", k=NCK)
                ntile = 2 * w + 2           # key tiles 0..2w+1
                # ---- scores + exp for all key tiles of this window
                pts = []
                for t in range(ntile):
                    st = ps_st.tile([128, 512], F32, tag="st", name=f"st{w}_{t}")
                    emit_st_mms(st, kt3, qt3, t)
                    pt = ptp.tile([128, 512], BF16, tag="pt", name=f"pt{w}_{t}")
                    nc.scalar.activation(pt[:], st[:], AFT.Exp, scale=SCALE)
                    if t >= 2 * w:
                        d = t - 2 * w
                        nc.vector.tensor_mul(pt[:], pt[:], mask_sb[:, 512 * d:512 * d + 512])
                    pts.append(pt)
                    # two windows of transpose lead; one window of qproj lead
                    if t == 0 and wi + 2 < NW:
                        xqt_cache[QORDER[wi + 2]] = emit_xq_tr(QORDER[wi + 2])
                if wi + 1 < NW:
                    qt_cache[QORDER[wi + 1]] = emit_qproj(
                        QORDER[wi + 1], xqt_cache.pop(QORDER[wi + 1]))
                # ---- attention @ V, two 256-query halves sequentially
                for al in range(2):
                    nt = ntile - 1 if al == 0 else ntile   # t=2w+1 is all-zero for al=0
                    acc = []
                    for s2 in range(2):
                        oa = ps_o.tile([128, 512], F32, tag=f"oa{s2}", name=f"oa{s2}_{w}_{al}")
                        ob = ps_o.tile([128, 258], F32, tag=f"ob{s2}", name=f"ob{s2}_{w}_{al}")
                        acc.append((oa, ob))
                    for t in range(nt):
                        for s2 in range(2):
                            qc = 256 * al + 128 * s2
                            oa, ob = acc[s2]
                            nc.tensor.matmul(oa[:], lhsT=pts[t][:, qc:qc + 128],
                                             rhs=v_b[:, 770 * t:770 * t + 512],
                                             start=(t == 0), stop=(t == nt - 1))
                            nc.tensor.matmul(ob[:], lhsT=pts[t][:, qc:qc + 128],
                                             rhs=v_b[:, 770 * t + 512:770 * t + 770],
                                             start=(t == 0), stop=(t == nt - 1))
                    for s2 in range(2):
                        oa, ob = acc[s2]
                        o_sb = obp.tile([128, 770], F32, tag="osb", name=f"osb{w}_{al}_{s2}")
                        nc.vector.tensor_copy(o_sb[:, 0:512], oa[:])
                        nc.scalar.activation(o_sb[:, 512:770], ob[:], AFT.Identity)
                        r0 = 512 * w + 256 * al + 128 * s2
                        nc.gpsimd.dma_start(out[r0: r0 + 128, :], o_sb[:, 0:769])

    nc.compile()
    return nc


def _build_masks(m):
    """Two diagonal masks for 512-query blocks, key tiles d=0,1 within the
    block: mask_d[j, ql] = (ql >= 256*d + 128*m + j).  [128, 1024] bf16."""
    jl = np.arange(128)[:, None]
    ql = np.arange(512)[None, :]
    out = np.empty((128, 1024), dtype=np.float32)
    for d in range(2):
        out[:, 512 * d:512 * d + 512] = (ql >= 256 * d + 128 * m + jl)
    return out.astype(ml_dtypes.bfloat16)


def kernel(input, Wq, bq, Wk, bk, Wv, bv):
    global last_exec_time_ns, last_results
    x = np.ascontiguousarray(np.asarray(input, dtype=np.float32))
    Wq = np.asarray(Wq, dtype=np.float32).astype(ml_dtypes.bfloat16)
    Wk = np.asarray(Wk, dtype=np.float32).astype(ml_dtypes.bfloat16)
    Wv = np.asarray(Wv, dtype=np.float32).astype(ml_dtypes.bfloat16)
    bq = np.ascontiguousarray(np.asarray(bq, dtype=np.float32))
    bk = np.ascontiguousarray(np.asarray(bk, dtype=np.float32))
    bv_np = np.ascontiguousarray(np.asarray(bv, dtype=np.float32))
    x_b = x.astype(ml_dtypes.bfloat16)

    if "nc" not in _nc_cache:
        _nc_cache["nc"] = build_module()
    nc = _nc_cache["nc"]

    masks = [_build_masks(m) for m in range(2)]
    key_rows = [np.concatenate([np.arange(128 * (2 * t + m), 128 * (2 * t + m) + 128)
                                for t in range(NKT)]) for m in range(2)]
    in_maps = []
    for core in range(8):
        b, m = core // 2, core % 2
        in_maps.append({
            "xq": x_b[b],
            "xk": np.ascontiguousarray(x_b[b][key_rows[m]]),
            "wqh": Wq, "wkh": Wk, "wvh": Wv, "bq": bq, "bk": bk,
            "msk": masks[m],
        })

    trace = bool(int(__import__("os").environ.get("KERNEL_TRACE", "0")))
    res = run_bass_kernel_spmd(nc, in_maps, core_ids=list(range(8)), trace=trace)
    last_exec_time_ns = res.exec_time_ns
    last_results = res

    y = np.empty((B, T, C), dtype=np.float32)
    for b in range(B):
        o0 = res.results[2 * b]["out"]
        o1 = res.results[2 * b + 1]["out"]
        O = o0[:, :C].astype(np.float64) + o1[:, :C].astype(np.float64)
        l = o0[:, C].astype(np.float64) + o1[:, C].astype(np.float64)
        y[b] = (O / l[:, None] + bv_np.astype(np.float64)).astype(np.float32)
    return y


# revision 6
# speedup vs baseline: 1.6619x; 1.3189x over previous
"""Trainium2 Bass kernel for single-head causal attention.

Problem: B=4, T=4096, C=768, fp32.
  Q = x@Wq+bq; K = x@Wk+bk; V = x@Wv+bv
  out = softmax(causal(Q K^T / sqrt(C))) @ V

Sharding (8 cores): 2 cores per batch element. Each core processes ALL 4096
queries of its batch but only HALF the key tiles (128-row tiles, interleaved
by parity m = core%2). Instruction streams are identical across cores (SPMD).

Score algebra (host folds the weights): softmax is invariant to per-row
constants, so with M = Wq Wk^T and wf = Wk bq,
  Q_i.K_j = x_i M x_j^T + x_j.wf + (row terms that cancel in softmax).
The per-key bias b_j = x_j.wf factors out of the softmax as f_j =
exp(SCALE*b_j), which is absorbed by scaling row j of V (and the
l-accumulator column) by f_j.  This removes the entire K projection: the
key-side score operand is just x^T (already produced by the DMA-transpose).

Each core returns unnormalized O_m = sum_j p_ij (f_j v_j) and l_m = sum_j
p_ij f_j.  Host combines:  out = (O_0 + O_1) / (l_0 + l_1) + bv.

Datatypes: no f32/f32r matmuls anywhere (f32r runs the PE in FP32_HIGH mode,
which disables fast weight load and roughly halves sustained throughput).
x, M, Wv ship as bf16; x is transposed during load by the DMA xbar (no PE
transposes).  XM projection and attention@V run bf16; XM^T and x^T are
rounded to fp8e4 and the score matmuls run fp8 DoubleRow (2 contraction
tiles per pass, ~2x bf16 rate).  Host-side softmax normalization absorbs the
fp8 score noise (~8e-3 worst-entry vs the 2e-2 gate).
"""
import sys

sys.path.insert(0, "/opt/trn_rl_repo")

import numpy as np
import ml_dtypes
from contextlib import ExitStack

import concourse.bass as bass
import concourse.bacc as bacc
import concourse.mybir as mybir
import concourse.tile as tile
from concourse.bass_utils import run_bass_kernel_spmd

dt = mybir.dt
F32, BF16, FP8 = dt.float32, dt.bfloat16, dt.float8e4
AFT = mybir.ActivationFunctionType
DR = mybir.MatmulPerfMode.DoubleRow

B, T, C = 4, 4096, 768
NCK = C // 128            # 6 contraction tiles
NKT = T // 2 // 128       # 16 key tiles per core
NW = T // 512             # 8 query/key windows of 512
SCALE = 1.0 / float(np.sqrt(np.float32(C)))
ST_FP8 = True             # False: scores in bf16 (more accurate, slower)

_nc_cache = {}
last_exec_time_ns = None
last_results = None


def build_module(st_fp8=ST_FP8):
    qkdt = FP8 if st_fp8 else BF16
    nc = bacc.Bacc("TRN2", target_bir_lowering=False, debug=False)

    xq = nc.dram_tensor("xq", [T, C], BF16, kind="ExternalInput").ap()
    xk = nc.dram_tensor("xk", [T // 2, C], BF16, kind="ExternalInput").ap()
    mh = nc.dram_tensor("mh", [C, C], BF16, kind="ExternalInput").ap()
    wvh = nc.dram_tensor("wvh", [C, C], BF16, kind="ExternalInput").ap()
    wf = nc.dram_tensor("wf", [C], BF16, kind="ExternalInput").ap()
    msk = nc.dram_tensor("msk", [128, 1024], BF16, kind="ExternalInput").ap()
    out = nc.dram_tensor("out", [T, C + 1], F32, kind="ExternalOutput").ap()

    with tile.TileContext(nc) as tc, ExitStack() as ctx:
        const = ctx.enter_context(tc.tile_pool(name="const", bufs=1))
        mask_sb = const.tile([128, 1024], BF16)
        wf_sb = const.tile([128, NCK], BF16)
        f_sb = const.tile([128, NKT], F32)      # per-key-tile softmax bias factor

        # persistent data
        w_pool = ctx.enter_context(tc.tile_pool(name="w", bufs=1))
        m_b = w_pool.tile([128, NCK * C], BF16)     # M planes [p, ck, co]
        wv_b = w_pool.tile([128, NCK * C], BF16)    # Wv planes [p, ck, co]
        xk8_pool = ctx.enter_context(tc.tile_pool(name="xk8", bufs=1))
        xk8 = xk8_pool.tile([128, NCK * 2048], qkdt)  # x^T planes [p, ck, key]
        v_pool = ctx.enter_context(tc.tile_pool(name="v", bufs=1))
        v_b = v_pool.tile([128, NKT * 770], BF16)     # per key tile [128, 770]

        xk83 = xk8[:].rearrange("p (k n) -> p k n", k=NCK)

        def emit_st_mms(stp, rhs3, t):
            """Score matmuls for key tile t: out [128 keys, 512 queries]."""
            if st_fp8:
                for j in range(NCK // 2):
                    nc.tensor.matmul(
                        stp[:],
                        lhsT=xk83[:, 2 * j:2 * j + 2, 128 * t:128 * t + 128],
                        rhs=rhs3[:, 2 * j:2 * j + 2, :],
                        perf_mode=DR, start=(j == 0), stop=(j == NCK // 2 - 1))
            else:
                for j in range(NCK):
                    nc.tensor.matmul(
                        stp[:],
                        lhsT=xk83[:, j, 128 * t:128 * t + 128],
                        rhs=rhs3[:, j, :],
                        start=(j == 0), stop=(j == NCK - 1))

        # ---------------- phase K: keys -> x^T(fp8), f, V(bf16, f-scaled) ---
        with tc.tile_pool(name="xkT", bufs=1) as xkTp, \
             tc.tile_pool(name="ps_k", bufs=2, space="PSUM") as ps_k:
            xkT_b = xkTp.tile([128, NCK * 2048], BF16)  # planes [p, ck, key]

            def emit_xk_tr(kw, eng):
                for k in range(NCK):
                    eng.dma_start(
                        xkT_b[:, 2048 * k + 512 * kw: 2048 * k + 512 * kw + 512],
                        xk[512 * kw: 512 * kw + 512, 128 * k: 128 * k + 128],
                        transpose=True)

            # startup: transposes first (sync + scalar rings in parallel),
            # weights on the software-DGE ring.
            emit_xk_tr(0, nc.sync)
            emit_xk_tr(1, nc.scalar)
            nc.gpsimd.dma_start(
                wv_b[:].rearrange("p (k n) -> p k n", k=NCK),
                wvh.rearrange("(k p) n -> p k n", p=128))
            nc.gpsimd.dma_start(wf_sb[:], wf.rearrange("(k p) -> p k", p=128))
            nc.gpsimd.dma_start(
                m_b[:].rearrange("p (k n) -> p k n", k=NCK),
                mh.rearrange("(k p) n -> p k n", p=128))
            nc.gpsimd.dma_start(mask_sb[:], msk[:])

            for kw in range(4):
                if kw + 2 < 4:
                    emit_xk_tr(kw + 2, nc.sync if kw == 0 else nc.scalar)
                # fp8 copy of this key window (score lhsT operand)
                nc.vector.tensor_copy(
                    xk83[:, :, 512 * kw:512 * kw + 512],
                    xkT_b[:].rearrange("p (k n) -> p k n", k=NCK)[:, :, 512 * kw:512 * kw + 512])
                for tt in range(4):
                    t_glob = 4 * kw + tt
                    # f_j = exp(SCALE * x_j . wf) for the 128 keys of this tile
                    facc = ps_k.tile([128, 1], F32, tag="facc")
                    for ck in range(NCK):
                        nc.tensor.matmul(
                            facc[:],
                            lhsT=xkT_b[:, 2048 * ck + 512 * kw + 128 * tt:
                                       2048 * ck + 512 * kw + 128 * tt + 128],
                            rhs=wf_sb[:, ck:ck + 1],
                            start=(ck == 0), stop=(ck == NCK - 1))
                    nc.scalar.activation(f_sb[:, t_glob:t_glob + 1], facc[:],
                                         AFT.Exp, scale=SCALE)
                    # V projection (bf16), f-scaled on eviction
                    pv1 = ps_k.tile([128, 512], F32, tag="pv1")
                    pv2 = ps_k.tile([128, 256], F32, tag="pv2")
                    for ck in range(NCK):
                        lt = xkT_b[:, 2048 * ck + 512 * kw + 128 * tt:
                                   2048 * ck + 512 * kw + 128 * tt + 128]
                        nc.tensor.matmul(pv1[:], lhsT=lt, rhs=wv_b[:, C * ck: C * ck + 512],
                                         start=(ck == 0), stop=(ck == NCK - 1))
                        nc.tensor.matmul(pv2[:], lhsT=lt, rhs=wv_b[:, C * ck + 512: C * ck + 768],
                                         start=(ck == 0), stop=(ck == NCK - 1))
                    fcol = f_sb[:, t_glob:t_glob + 1]
                    nc.vector.tensor_scalar_mul(v_b[:, 770 * t_glob: 770 * t_glob + 512],
                                                pv1[:], fcol)
                    nc.vector.tensor_scalar_mul(v_b[:, 770 * t_glob + 512: 770 * t_glob + 768],
                                                pv2[:], fcol)
                    nc.vector.tensor_copy(v_b[:, 770 * t_glob + 768: 770 * t_glob + 769], fcol)
                    nc.vector.memset(v_b[:, 770 * t_glob + 769: 770 * t_glob + 770], 0.0)

        # ---------------- phase Q: flash over 512-query windows ----------------
        ps_pj = ctx.enter_context(tc.tile_pool(name="ps_pj", bufs=2, space="PSUM"))
        ps_st = ctx.enter_context(tc.tile_pool(name="ps_st", bufs=2, space="PSUM"))
        ps_o = ctx.enter_context(tc.tile_pool(name="ps_o", bufs=1, space="PSUM"))
        with tc.tile_pool(name="xqst", bufs=3) as xqst, \
             tc.tile_pool(name="qt", bufs=2) as qtp, \
             tc.tile_pool(name="pt", bufs=16) as ptp, \
             tc.tile_pool(name="ob", bufs=2) as obp:

            QORDER = list(range(NW - 1, -1, -1))  # big windows first

            def emit_xq_tr(w):
                xqt = xqst.tile([128, NCK * 512], BF16, tag="xqt", name=f"xqt{w}")
                for k in range(NCK):
                    nc.sync.dma_start(
                        xqt[:, 512 * k: 512 * k + 512],
                        xq[512 * w: 512 * w + 512, 128 * k: 128 * k + 128],
                        transpose=True)
                return xqt

            def emit_qproj(w, xqt):
                """XM^T for window w: planes [p, co, 512] in qkdt."""
                qt_sb = qtp.tile([128, NCK * 512], qkdt, tag="qt", name=f"qt{w}")
                qt3 = qt_sb[:].rearrange("p (k n) -> p k n", k=NCK)
                for co in range(NCK):
                    pj = ps_pj.tile([128, 512], F32, tag="pj")
                    for ck in range(NCK):
                        nc.tensor.matmul(
                            pj[:],
                            lhsT=m_b[:, C * ck + 128 * co: C * ck + 128 * co + 128],
                            rhs=xqt[:, 512 * ck: 512 * ck + 512],
                            start=(ck == 0), stop=(ck == NCK - 1))
                    nc.scalar.activation(qt3[:, co, :], pj[:], AFT.Identity)
                return qt_sb

            # stage the first two windows' transposes + first window's proj
            xqt_cache = {QORDER[0]: emit_xq_tr(QORDER[0]),
                         QORDER[1]: emit_xq_tr(QORDER[1])}
            qt_cache = {QORDER[0]: emit_qproj(QORDER[0], xqt_cache.pop(QORDER[0]))}

            for wi, w in enumerate(QORDER):
                qt_sb = qt_cache.pop(w)
                qt3 = qt_sb[:].rearrange("p (k n) -> p k n", k=NCK)
                ntile = 2 * w + 2           # key tiles 0..2w+1

                pts = {}

                def do_st(t):
                    st = ps_st.tile([128, 512], F32, tag="st", name=f"st{w}_{t}")
                    emit_st_mms(st, qt3, t)
                    pt = ptp.tile([128, 512], BF16, tag="pt", name=f"pt{w}_{t}")
                    nc.scalar.activation(pt[:], st[:], AFT.Exp, scale=SCALE)
                    if t >= 2 * w:
                        d = t - 2 * w
                        nc.vector.tensor_mul(pt[:], pt[:], mask_sb[:, 512 * d:512 * d + 512])
                    pts[t] = pt

                def av_mms(al, t, nt):
                    for s2 in range(2):
                        qc = 256 * al + 128 * s2
                        oa, ob = acc[s2]
                        nc.tensor.matmul(oa[:], lhsT=pts[t][:, qc:qc + 128],
                                         rhs=v_b[:, 770 * t:770 * t + 512],
                                         start=(t == 0), stop=(t == nt - 1))
                        nc.tensor.matmul(ob[:], lhsT=pts[t][:, qc:qc + 128],
                                         rhs=v_b[:, 770 * t + 512:770 * t + 770],
                                         start=(t == 0), stop=(t == nt - 1))

                def drain(al):
                    for s2 in range(2):
                        oa, ob = acc[s2]
                        o_sb = obp.tile([128, 770], F32, tag="osb", name=f"osb{w}_{al}_{s2}")
                        nc.vector.tensor_copy(o_sb[:, 0:512], oa[:])
                        nc.scalar.activation(o_sb[:, 512:770], ob[:], AFT.Identity)
                        r0 = 512 * w + 256 * al + 128 * s2
                        eng = nc.sync if w <= 1 else nc.gpsimd
                        eng.dma_start(out[r0: r0 + 128, :], o_sb[:, 0:769])

                # scores pipelined two tiles ahead of the al=0 accumulation
                do_st(0)
                if ntile > 1:
                    do_st(1)
                if wi + 2 < NW:
                    xqt_cache[QORDER[wi + 2]] = emit_xq_tr(QORDER[wi + 2])
                acc = []
                for s2 in range(2):
                    oa = ps_o.tile([128, 512], F32, tag=f"oa{s2}", name=f"oa{s2}_{w}_0")
                    ob = ps_o.tile([128, 258], F32, tag=f"ob{s2}", name=f"ob{s2}_{w}_0")
                    acc.append((oa, ob))
                nt0 = ntile - 1             # t=2w+1 is all-masked for al=0
                for t in range(nt0):
                    if t + 2 < ntile:
                        do_st(t + 2)
                    av_mms(0, t, nt0)
                drain(0)
                # al=1 burst; next window's projection rides along here
                acc = []
                for s2 in range(2):
                    oa = ps_o.tile([128, 512], F32, tag=f"oa{s2}", name=f"oa{s2}_{w}_1")
                    ob = ps_o.tile([128, 258], F32, tag=f"ob{s2}", name=f"ob{s2}_{w}_1")
                    acc.append((oa, ob))
                for t in range(ntile):
                    av_mms(1, t, ntile)
                    if t == 0 and wi + 1 < NW:
                        qt_cache[QORDER[wi + 1]] = emit_qproj(
                            QORDER[wi + 1], xqt_cache.pop(QORDER[wi + 1]))
                drain(1)

    nc.compile()
    return nc


def _build_masks(m):
    """Two diagonal masks for 512-query blocks, key tiles d=0,1 within the
    block: mask_d[j, ql] = (ql >= 256*d + 128*m + j).  [128, 1024] bf16."""
    jl = np.arange(128)[:, None]
    ql = np.arange(512)[None, :]
    out = np.empty((128, 1024), dtype=np.float32)
    for d in range(2):
        out[:, 512 * d:512 * d + 512] = (ql >= 256 * d + 128 * m + jl)
    return out.astype(ml_dtypes.bfloat16)


def kernel(input, Wq, bq, Wk, bk, Wv, bv):
    global last_exec_time_ns, last_results
    x = np.ascontiguousarray(np.asarray(input, dtype=np.float32))
    Wq = np.asarray(Wq, dtype=np.float32)
    Wk = np.asarray(Wk, dtype=np.float32)
    Wv = np.asarray(Wv, dtype=np.float32)
    bq = np.asarray(bq, dtype=np.float32)
    bv_np = np.ascontiguousarray(np.asarray(bv, dtype=np.float32))
    M = (Wq @ Wk.T).astype(ml_dtypes.bfloat16)
    wf = (Wk @ bq).astype(ml_dtypes.bfloat16)
    Wv_b = Wv.astype(ml_dtypes.bfloat16)
    x_b = x.astype(ml_dtypes.bfloat16)

    if "nc" not in _nc_cache:
        _nc_cache["nc"] = build_module()
    nc = _nc_cache["nc"]

    masks = [_build_masks(m) for m in range(2)]
    key_rows = [np.concatenate([np.arange(128 * (2 * t + m), 128 * (2 * t + m) + 128)
                                for t in range(NKT)]) for m in range(2)]
    in_maps = []
    for core in range(8):
        b, m = core // 2, core % 2
        in_maps.append({
            "xq": x_b[b],
            "xk": np.ascontiguousarray(x_b[b][key_rows[m]]),
            "mh": M, "wvh": Wv_b, "wf": wf,
            "msk": masks[m],
        })

    trace = bool(int(__import__("os").environ.get("KERNEL_TRACE", "0")))
    res = run_bass_kernel_spmd(nc, in_maps, core_ids=list(range(8)), trace=trace)
    last_exec_time_ns = res.exec_time_ns
    last_results = res

    y = np.empty((B, T, C), dtype=np.float32)
    for b in range(B):
        o0 = res.results[2 * b]["out"]
        o1 = res.results[2 * b + 1]["out"]
        O = o0[:, :C].astype(np.float64) + o1[:, :C].astype(np.float64)
        l = o0[:, C].astype(np.float64) + o1[:, C].astype(np.float64)
        y[b] = (O / l[:, None] + bv_np.astype(np.float64)).astype(np.float32)
    return y
